# revision 10
# baseline (speedup 1.0000x reference)
"""Per-core causal multi-head attention Bass/Tile program builder.

One core handles: batch b, one head-group (DH of the model's head dims).
Computation (all on-chip after the initial loads, fp32r matmuls):
  qT = wqT.T @ xT          [DH, S]   (head dims on partitions)
  kT = wkT.T @ xT          [DH, S]
  v  = xT.T @ wvT          [S, DH]   (+ a ones column per head for softmax denom)
  per head pair p (2 heads stacked on 128 partitions):
    scoresT[sk, q] = kT.T @ qT    (K=64 contraction per head, heads packed in
                                   row strips 0-63 / 64-127 of the PE array)
    expT = exp(0.125 * scoresT)   (ACT, one [128,1024] op for the pair)
    expT *= causal mask           (diagonal tiles only)
    avT[65, q] += [v|1].T @ expT  (row 64 accumulates the softmax denominator)
    avT[0:64] *= 1/denom          (K=1 ones-matmul broadcasts the reciprocal row)
  out[s, :] = avT.T @ woT       (accumulated over head pairs, written to HBM)
"""

from contextlib import ExitStack

import numpy as np

import concourse.bass as bass
import concourse.bacc as bacc
import concourse.mybir as mybir
import concourse.tile as tile

F32 = mybir.dt.float32
F32R = mybir.dt.float32r


def r(ap):
    """View an fp32 AP as float32r for full-rate PE matmuls."""
    return ap.bitcast(F32R)


def make_masks(n_j=4, qb=512, extra_ones=64):
    """[128, n_j*qb + extra_ones] fp32: causal 0/1 masks for the n_j diagonal
    block offsets, plus a strip of ones (bcast-matmul lhsT / v ones source)."""
    p = np.arange(128)[:, None]
    f = np.arange(qb)[None, :]
    cols = [((p + 128 * j) <= f).astype(np.float32) for j in range(n_j)]
    cols.append(np.ones((128, extra_ones), np.float32))
    return np.concatenate(cols, axis=1)


def build_core_program(S=2048, D=1024, DH=512, DOUT=1024, QB=512, debug=False):
    """Build the per-core Bass program. Returns nc."""
    P = 128
    HP = DH // P            # head pairs
    H = DH // 64            # heads on this core
    ND = D // P             # d tiles
    NS = S // P             # s tiles of 128
    NQB = S // QB           # q blocks
    NSB = S // 512          # s blocks of 512 (projection free blocks)
    NJ = QB // P            # diagonal offsets per q block
    MCOLS = NJ * QB + 64    # masks input width

    nc = bacc.Bacc()

    xT = nc.dram_tensor("xT", [D, S], F32R, kind="ExternalInput")
    wqT = nc.dram_tensor("wqT", [D, DH], F32R, kind="ExternalInput")
    wkT = nc.dram_tensor("wkT", [D, DH], F32R, kind="ExternalInput")
    wvT = nc.dram_tensor("wvT", [D, DH], F32R, kind="ExternalInput")
    woT = nc.dram_tensor("woT", [DH, DOUT], F32R, kind="ExternalInput")
    masks = nc.dram_tensor("masks", [P, MCOLS], F32R, kind="ExternalInput")
    out = nc.dram_tensor("out", [S, DOUT], F32, kind="ExternalOutput")
    if debug:
        dbg_qT = nc.dram_tensor("dbg_qT", [P, S], F32, kind="ExternalOutput")
        dbg_kT = nc.dram_tensor("dbg_kT", [P, S], F32, kind="ExternalOutput")
        dbg_v = nc.dram_tensor("dbg_v", [P, H * 65], F32, kind="ExternalOutput")
        dbg_avT = nc.dram_tensor("dbg_avT", [P, S], F32, kind="ExternalOutput")
        dbg_ex = nc.dram_tensor("dbg_ex", [P, 1024], F32, kind="ExternalOutput")

    lp = nc.allow_low_precision(reason="float32r is bitwise float32 on every non-PE datapath")
    with lp, tile.TileContext(nc) as tc, ExitStack() as ctx:
        const_pool = ctx.enter_context(tc.tile_pool(name="const", bufs=1))
        # x half-tiles and avT share slots (x is dead before avT allocates)
        big_pool = ctx.enter_context(tc.tile_pool(name="big", bufs=ND // 2 + 1))
        qk_pool = ctx.enter_context(tc.tile_pool(name="qk", bufs=2 * HP))
        v_pool = ctx.enter_context(tc.tile_pool(name="v", bufs=NS))
        w_pool = ctx.enter_context(tc.tile_pool(name="w", bufs=3 * (ND // 2) + 2))
        e_pool = ctx.enter_context(tc.tile_pool(name="e1024", bufs=2 + HP))
        o_pool = ctx.enter_context(tc.tile_pool(name="outp", bufs=5))
        # one PSUM pool, 8 banks total: ps 2 + sc 2x2 + av 2 = 8 (bc shares "ps")
        psum_pool = ctx.enter_context(tc.tile_pool(name="psum", bufs=1, space="PSUM"))

        mask_t = const_pool.tile([P, MCOLS], F32R, tag="masks")
        nc.sync.dma_start(mask_t[:], masks[:, :])
        ones64 = mask_t[:, NJ * QB : NJ * QB + 64]  # all-ones [128, 64]

        # persistent activations
        qT = [qk_pool.tile([P, S], F32R, tag="qk", name="qT") for _ in range(HP)]
        kT = [qk_pool.tile([P, S], F32R, tag="qk", name="kT") for _ in range(HP)]
        v_t = [v_pool.tile([P, H * 65], F32R, tag="v", name="v_t") for _ in range(NS)]
        woT_t = [e_pool.tile([P, DOUT], F32R, tag="e1024", name="woT_t") for _ in range(HP)]
        for p in range(HP):
            nc.sync.dma_start(woT_t[p][:], woT[p * P : (p + 1) * P, :])

        # ---- projections, d contracted in two halves to bound SBUF ----
        HALF = ND // 2
        for half in range(2):
            x_t = [big_pool.tile([P, S], F32R, tag="big", name="x_t") for _ in range(HALF)]
            for i in range(HALF):
                dt = half * HALF + i
                nc.sync.dma_start(x_t[i][:], xT[dt * P : (dt + 1) * P, :])

            # wv first: v-proj runs before qk-proj, so its weights must not
            # wait on wq/wk pool slots (allocation order = emission order)
            wv_t = [w_pool.tile([P, DH], F32R, tag="w", name="wv_t") for _ in range(HALF)]
            for i in range(HALF):
                dt = half * HALF + i
                nc.sync.dma_start(wv_t[i][:], wvT[dt * P : (dt + 1) * P, :])
            wq_t = [w_pool.tile([P, DH], F32R, tag="w", name="wq_t") for _ in range(HALF)]
            wk_t = [w_pool.tile([P, DH], F32R, tag="w", name="wk_t") for _ in range(HALF)]
            for i in range(HALF):
                dt = half * HALF + i
                nc.sync.dma_start(wq_t[i][:], wqT[dt * P : (dt + 1) * P, :])
                nc.sync.dma_start(wk_t[i][:], wkT[dt * P : (dt + 1) * P, :])

            # v first (attention needs all of v before any pair starts):
            # out [s-tile 128, DH] -> strided per-head (64 cols + ones col)
            for st in range(NS):
                ps = psum_pool.tile([P, 512], F32, tag="ps", name="ps_v", bufs=2)[:, :DH]
                for i in range(HALF):
                    nc.tensor.matmul(
                        ps[:],
                        r(x_t[i][:, st * P : (st + 1) * P]),
                        r(wv_t[i][:]),
                        start=(i == 0),
                        stop=(i == HALF - 1),
                    )
                dst = v_t[st][:].rearrange("p (h c) -> p h c", c=65)[:, :, 0:64]
                src = ps[:].rearrange("p (h c) -> p h c", c=64)
                if half == 0:
                    nc.vector.tensor_copy(dst, src)
                else:
                    nc.vector.tensor_add(dst, dst, src)
            if half == 1:
                for st in range(NS):
                    onescol = v_t[st][:].rearrange("p (h c) -> p h c", c=65)[:, :, 64:65]
                    nc.vector.tensor_copy(
                        onescol, ones64[:, 0:H].rearrange("p (h c) -> p h c", c=1)
                    )

            # q/k pair-major so attention on pair 0 can start early
            for p in range(HP):
                for w_t, dst in ((wq_t, qT), (wk_t, kT)):
                    for sb in range(NSB):
                        ps = psum_pool.tile([P, 512], F32, tag="ps", bufs=2)
                        for i in range(HALF):
                            nc.tensor.matmul(
                                ps[:],
                                r(w_t[i][:, p * P : (p + 1) * P]),
                                r(x_t[i][:, sb * 512 : (sb + 1) * 512]),
                                start=(i == 0),
                                stop=(i == HALF - 1),
                            )
                        sl = dst[p][:, sb * 512 : (sb + 1) * 512]
                        if half == 0:
                            nc.vector.tensor_copy(sl, ps[:])
                        else:
                            nc.vector.tensor_add(sl, sl, ps[:])

        # ---- attention ----
        # Software-pipelined: scores for step sk+1 are issued to the PE FIFO
        # before the av matmuls of step sk, so the in-order PE never sits
        # behind an av matmul that is still waiting on the ACT exp.
        avT = [big_pool.tile([P, S], F32R, tag="big", name="avT") for _ in range(HP)]
        for p in range(HP):
            hA, hB = 2 * p, 2 * p + 1
            for qb in range(NQB):
                Q0 = qb * QB
                av_ps = [psum_pool.tile([65, 512], F32, tag="av", name="av_ps", bufs=2) for _ in range(2)]
                nsk = (Q0 + QB) // P

                def scores(sk):
                    K0 = sk * P
                    j = sk - NJ * qb
                    # causal: q columns f < 128j of this block can't attend
                    # to this k tile — skip them in scores/exp/av entirely.
                    c0 = max(0, j) * P
                    scp = []
                    for hi in range(2):
                        sc = psum_pool.tile(
                            [P, 512], F32, tag=f"sc{hi}", name=f"sc{hi}", bufs=2
                        )
                        nc.tensor.matmul(
                            sc[:, c0:512],
                            r(kT[p][hi * 64 : hi * 64 + 64, K0 : K0 + P]),
                            r(qT[p][hi * 64 : hi * 64 + 64, Q0 + c0 : Q0 + QB]),
                        )
                        scp.append(sc)
                    return scp

                def tail(sk, scp):
                    j = sk - NJ * qb
                    c0 = max(0, j) * P
                    ex = e_pool.tile([P, 1024], F32R, tag="e1024", name="ex")
                    for hi in range(2):
                        nc.scalar.activation(
                            ex[:, hi * 512 + c0 : hi * 512 + 512],
                            scp[hi][:, c0:512],
                            mybir.ActivationFunctionType.Exp,
                            scale=0.125,
                        )
                    if j >= 0:  # diagonal strip [c0, c0+128): triangular mask
                        m128 = mask_t[:, 0:P]
                        for hi in range(2):
                            nc.vector.tensor_mul(
                                ex[:, hi * 512 + c0 : hi * 512 + c0 + P],
                                ex[:, hi * 512 + c0 : hi * 512 + c0 + P],
                                m128,
                            )
                    for hi, h in enumerate((hA, hB)):
                        nc.tensor.matmul(
                            av_ps[hi][:, c0:512],
                            r(v_t[sk][:, h * 65 : h * 65 + 65]),
                            r(ex[:, hi * 512 + c0 : hi * 512 + 512]),
                            start=(sk == 0),
                            stop=(sk == nsk - 1),
                        )

                pend = None
                for sk in range(nsk):
                    scp = scores(sk)
                    if pend is not None:
                        tail(*pend)
                    pend = (sk, scp)
                tail(*pend)

                # softmax normalization: row 64 of av_ps holds the denominator.
                # Entirely off the PE: DVE copy, GpSimd partition broadcast,
                # fast-approx reciprocal, multiply.
                for hi in range(2):
                    dn = o_pool.tile([P, 512], F32, tag="outp", name="dn")
                    nc.vector.tensor_copy(dn[64:65, :], av_ps[hi][64:65, :])
                    bcd = o_pool.tile([P, 512], F32, tag="outp", name="bcd")
                    nc.gpsimd.partition_broadcast(bcd[0:64, :], dn[64:65, :])
                    rcb = o_pool.tile([P, 512], F32, tag="outp", name="rcb")
                    nc.vector.reciprocal_approx_fast(out=rcb[0:64, :], in_=bcd[0:64, :])
                    if hi == 0:
                        nc.vector.tensor_mul(
                            avT[p][0:64, Q0 : Q0 + QB], av_ps[hi][0:64, :], rcb[0:64, :]
                        )
                    else:
                        tmp = o_pool.tile([P, 512], F32R, tag="outp")
                        nc.vector.tensor_mul(tmp[0:64, :], av_ps[hi][0:64, :], rcb[0:64, :])
                        # partition shift 0:64 -> 64:128 (engines can't cross lanes)
                        nc.sync.dma_start(avT[p][64:128, Q0 : Q0 + QB], tmp[0:64, :])

        if debug:
            nc.sync.dma_start(dbg_qT[:, :], qT[0][:])
            nc.sync.dma_start(dbg_kT[:, :], kT[0][:])
            nc.sync.dma_start(dbg_v[:, :], v_t[0][:])
            nc.sync.dma_start(dbg_avT[:, :], avT[0][:])

        # ---- output projection: out[s, n] = sum_p avT[p].T @ woT[p] ----
        NW = min(512, DOUT)
        for st in range(NS):
            for nb in range(DOUT // NW):
                ps = psum_pool.tile([P, 512], F32, tag="ps", name="ps_o", bufs=2)
                for p in range(HP):
                    nc.tensor.matmul(
                        ps[:, :NW],
                        r(avT[p][:, st * P : (st + 1) * P]),
                        r(woT_t[p][:, nb * NW : (nb + 1) * NW]),
                        start=(p == 0),
                        stop=(p == HP - 1),
                    )
                ot = o_pool.tile([P, 512], F32, tag="outp", name="ot")
                nc.vector.tensor_copy(ot[:, :NW], ps[:, :NW])
                nc.sync.dma_start(
                    out[st * P : (st + 1) * P, nb * NW : (nb + 1) * NW], ot[:, :NW]
                )

    nc.compile()
    return nc


def shard_inputs(x, wq, wk, wv, wo, n_cores=8):
    """Full inputs -> per-core in_maps. Core c: batch c//2, head-group c%2."""
    B = x.shape[0]
    D = wq.shape[1]
    hg_w = wq.shape[0] // (n_cores // B)
    masks = make_masks()
    in_maps = []
    for c in range(n_cores):
        b, hg = c // (n_cores // B), c % (n_cores // B)
        sl = slice(hg * hg_w, (hg + 1) * hg_w)
        in_maps.append(
            {
                "xT": np.ascontiguousarray(x[b].T),
                "wqT": np.ascontiguousarray(wq[sl, :].T),
                "wkT": np.ascontiguousarray(wk[sl, :].T),
                "wvT": np.ascontiguousarray(wv[sl, :].T),
                "woT": np.ascontiguousarray(wo[:, sl].T),
                "masks": masks,
            }
        )
    return in_maps


def unshard_outputs(results, B=4):
    """Per-core 'out' partials -> full [B, S, D] output (sum head-group pairs)."""
    per_b = len(results) // B
    outs = []
    for b in range(B):
        acc = results[b * per_b]["out"].astype(np.float32)
        for i in range(1, per_b):
            acc = acc + results[b * per_b + i]["out"]
        outs.append(acc)
    return np.stack(outs, axis=0)


# ---------------------------------------------------------------------------
# Full-kernel entry point: FULL inputs -> FULL output, 8 NeuronCores.
# Sharding: core c -> (batch c//2, head-group c%2). Each core computes its
# batch's attention for 8 of the 16 heads plus that head-group's slice of the
# output projection; the two partial outputs per batch are summed on the host
# (the unshard step of the tensor-parallel split of wo).
# ---------------------------------------------------------------------------

_NC_CACHE = {}


def _get_program():
    if "nc" not in _NC_CACHE:
        _NC_CACHE["nc"] = build_core_program(S=2048, D=1024, DH=512, DOUT=1024)
    return _NC_CACHE["nc"]


def kernel(x, wq, wk, wv, wo):
    from concourse import bass_utils

    x = np.asarray(x, dtype=np.float32)
    wq = np.asarray(wq, dtype=np.float32)
    wk = np.asarray(wk, dtype=np.float32)
    wv = np.asarray(wv, dtype=np.float32)
    wo = np.asarray(wo, dtype=np.float32)

    nc = _get_program()
    in_maps = shard_inputs(x, wq, wk, wv, wo, n_cores=8)
    res = bass_utils.run_bass_kernel_spmd(nc, in_maps, core_ids=list(range(8)))
    return unshard_outputs(res.results, B=x.shape[0])


# revision 11
# speedup vs baseline: 1.0483x; 1.0483x over previous
"""Per-core causal multi-head attention Bass/Tile program builder.

One core handles: batch b, one head-group (DH of the model's head dims).
Computation (all on-chip after the initial loads, fp32r matmuls):
  qT = wqT.T @ xT          [DH, S]   (head dims on partitions)
  kT = wkT.T @ xT          [DH, S]
  v  = xT.T @ wvT          [S, DH]   (+ a ones column per head for softmax denom)
  per head pair p (2 heads stacked on 128 partitions):
    scoresT[sk, q] = kT.T @ qT    (K=64 contraction per head, heads packed in
                                   row strips 0-63 / 64-127 of the PE array)
    expT = exp(0.125 * scoresT)   (ACT, one [128,1024] op for the pair)
    expT *= causal mask           (diagonal tiles only)
    avT[65, q] += [v|1].T @ expT  (row 64 accumulates the softmax denominator)
    avT[0:64] *= 1/denom          (K=1 ones-matmul broadcasts the reciprocal row)
  out[s, :] = avT.T @ woT       (accumulated over head pairs, written to HBM)
"""

from contextlib import ExitStack

import numpy as np

import concourse.bass as bass
import concourse.bacc as bacc
import concourse.mybir as mybir
import concourse.tile as tile

F32 = mybir.dt.float32
F32R = mybir.dt.float32r


def r(ap):
    """View an fp32 AP as float32r for full-rate PE matmuls."""
    return ap.bitcast(F32R)


def make_masks(n_j=4, qb=512, extra_ones=64):
    """[128, n_j*qb + extra_ones] fp32: causal 0/1 masks for the n_j diagonal
    block offsets, plus a strip of ones (bcast-matmul lhsT / v ones source)."""
    p = np.arange(128)[:, None]
    f = np.arange(qb)[None, :]
    cols = [((p + 128 * j) <= f).astype(np.float32) for j in range(n_j)]
    cols.append(np.ones((128, extra_ones), np.float32))
    return np.concatenate(cols, axis=1)


def build_core_program(S=2048, D=1024, DH=512, DOUT=1024, QB=512, debug=False):
    """Build the per-core Bass program. Returns nc."""
    P = 128
    HP = DH // P            # head pairs
    H = DH // 64            # heads on this core
    ND = D // P             # d tiles
    NS = S // P             # s tiles of 128
    NQB = S // QB           # q blocks
    NSB = S // 512          # s blocks of 512 (projection free blocks)
    NJ = QB // P            # diagonal offsets per q block
    MCOLS = NJ * QB + 64    # masks input width

    nc = bacc.Bacc()

    xT = nc.dram_tensor("xT", [D, S], F32R, kind="ExternalInput")
    wqT = nc.dram_tensor("wqT", [D, DH], F32R, kind="ExternalInput")
    wkT = nc.dram_tensor("wkT", [D, DH], F32R, kind="ExternalInput")
    wvT = nc.dram_tensor("wvT", [D, DH], F32R, kind="ExternalInput")
    woT = nc.dram_tensor("woT", [DH, DOUT], F32R, kind="ExternalInput")
    masks = nc.dram_tensor("masks", [P, MCOLS], F32R, kind="ExternalInput")
    out = nc.dram_tensor("out", [S, DOUT], F32, kind="ExternalOutput")
    if debug:
        dbg_qT = nc.dram_tensor("dbg_qT", [P, S], F32, kind="ExternalOutput")
        dbg_kT = nc.dram_tensor("dbg_kT", [P, S], F32, kind="ExternalOutput")
        dbg_v = nc.dram_tensor("dbg_v", [P, H * 65], F32, kind="ExternalOutput")
        dbg_avT = nc.dram_tensor("dbg_avT", [P, S], F32, kind="ExternalOutput")
        dbg_ex = nc.dram_tensor("dbg_ex", [P, 1024], F32, kind="ExternalOutput")

    lp = nc.allow_low_precision(reason="float32r is bitwise float32 on every non-PE datapath")
    with lp, tile.TileContext(nc) as tc, ExitStack() as ctx:
        const_pool = ctx.enter_context(tc.tile_pool(name="const", bufs=1))
        # x half-tiles and avT share slots (x is dead before avT allocates)
        big_pool = ctx.enter_context(tc.tile_pool(name="big", bufs=ND // 2 + 1))
        qk_pool = ctx.enter_context(tc.tile_pool(name="qk", bufs=2 * HP))
        v_pool = ctx.enter_context(tc.tile_pool(name="v", bufs=NS))
        w_pool = ctx.enter_context(tc.tile_pool(name="w", bufs=3 * (ND // 2) + 2))
        e_pool = ctx.enter_context(tc.tile_pool(name="e1024", bufs=2 + HP))
        o_pool = ctx.enter_context(tc.tile_pool(name="outp", bufs=5))
        # one PSUM pool, 8 banks total: ps 2 + sc 2x2 + av 2 = 8 (bc shares "ps")
        psum_pool = ctx.enter_context(tc.tile_pool(name="psum", bufs=1, space="PSUM"))

        mask_t = const_pool.tile([P, MCOLS], F32R, tag="masks")
        nc.sync.dma_start(mask_t[:], masks[:, :])
        ones64 = mask_t[:, NJ * QB : NJ * QB + 64]  # all-ones [128, 64]

        # persistent activations
        qT = [qk_pool.tile([P, S], F32R, tag="qk", name="qT") for _ in range(HP)]
        kT = [qk_pool.tile([P, S], F32R, tag="qk", name="kT") for _ in range(HP)]
        v_t = [v_pool.tile([P, H * 65], F32R, tag="v", name="v_t") for _ in range(NS)]
        woT_t = [e_pool.tile([P, DOUT], F32R, tag="e1024", name="woT_t") for _ in range(HP)]
        for p in range(HP):
            nc.sync.dma_start(woT_t[p][:], woT[p * P : (p + 1) * P, :])

        # ---- projections, d contracted in two halves to bound SBUF ----
        HALF = ND // 2
        for half in range(2):
            x_t = [big_pool.tile([P, S], F32R, tag="big", name="x_t") for _ in range(HALF)]
            for i in range(HALF):
                dt = half * HALF + i
                nc.sync.dma_start(x_t[i][:], xT[dt * P : (dt + 1) * P, :])

            # wv first: v-proj runs before qk-proj, so its weights must not
            # wait on wq/wk pool slots (allocation order = emission order)
            wv_t = [w_pool.tile([P, DH], F32R, tag="w", name="wv_t") for _ in range(HALF)]
            for i in range(HALF):
                dt = half * HALF + i
                nc.sync.dma_start(wv_t[i][:], wvT[dt * P : (dt + 1) * P, :])
            wq_t = [w_pool.tile([P, DH], F32R, tag="w", name="wq_t") for _ in range(HALF)]
            wk_t = [w_pool.tile([P, DH], F32R, tag="w", name="wk_t") for _ in range(HALF)]
            for i in range(HALF):
                dt = half * HALF + i
                nc.sync.dma_start(wq_t[i][:], wqT[dt * P : (dt + 1) * P, :])
                nc.sync.dma_start(wk_t[i][:], wkT[dt * P : (dt + 1) * P, :])

            # v first (attention needs all of v before any pair starts):
            # out [s-tile 128, DH] -> strided per-head (64 cols + ones col)
            for st in range(NS):
                ps = psum_pool.tile([P, 512], F32, tag="ps", name="ps_v", bufs=2)[:, :DH]
                for i in range(HALF):
                    nc.tensor.matmul(
                        ps[:],
                        r(x_t[i][:, st * P : (st + 1) * P]),
                        r(wv_t[i][:]),
                        start=(i == 0),
                        stop=(i == HALF - 1),
                    )
                dst = v_t[st][:].rearrange("p (h c) -> p h c", c=65)[:, :, 0:64]
                src = ps[:].rearrange("p (h c) -> p h c", c=64)
                if half == 0:
                    nc.vector.tensor_copy(dst, src)
                else:
                    nc.vector.tensor_add(dst, dst, src)
            if half == 1:
                for st in range(NS):
                    onescol = v_t[st][:].rearrange("p (h c) -> p h c", c=65)[:, :, 64:65]
                    nc.vector.tensor_copy(
                        onescol, ones64[:, 0:H].rearrange("p (h c) -> p h c", c=1)
                    )

            # q/k pair-major so attention on pair 0 can start early
            for p in range(HP):
                for w_t, dst in ((wq_t, qT), (wk_t, kT)):
                    for sb in range(NSB):
                        ps = psum_pool.tile([P, 512], F32, tag="ps", bufs=2)
                        for i in range(HALF):
                            nc.tensor.matmul(
                                ps[:],
                                r(w_t[i][:, p * P : (p + 1) * P]),
                                r(x_t[i][:, sb * 512 : (sb + 1) * 512]),
                                start=(i == 0),
                                stop=(i == HALF - 1),
                            )
                        sl = dst[p][:, sb * 512 : (sb + 1) * 512]
                        if half == 0:
                            nc.vector.tensor_copy(sl, ps[:])
                        else:
                            nc.vector.tensor_add(sl, sl, ps[:])

        # ---- attention ----
        # Software-pipelined: scores for step sk+1 are issued to the PE FIFO
        # before the av matmuls of step sk, so the in-order PE never sits
        # behind an av matmul that is still waiting on the ACT exp.
        avT = [big_pool.tile([P, S], F32R, tag="big", name="avT") for _ in range(HP)]
        for p in range(HP):
            hA, hB = 2 * p, 2 * p + 1
            for qb in range(NQB):
                Q0 = qb * QB
                av_ps = [psum_pool.tile([65, 512], F32, tag="av", name="av_ps", bufs=2) for _ in range(2)]
                nsk = (Q0 + QB) // P

                def scores(sk):
                    K0 = sk * P
                    j = sk - NJ * qb
                    # causal: q columns f < 128j of this block can't attend
                    # to this k tile — skip them in scores/exp/av entirely.
                    c0 = max(0, j) * P
                    scp = []
                    for hi in range(2):
                        sc = psum_pool.tile(
                            [P, 512], F32, tag=f"sc{hi}", name=f"sc{hi}", bufs=2
                        )
                        nc.tensor.matmul(
                            sc[:, c0:512],
                            r(kT[p][hi * 64 : hi * 64 + 64, K0 : K0 + P]),
                            r(qT[p][hi * 64 : hi * 64 + 64, Q0 + c0 : Q0 + QB]),
                        )
                        scp.append(sc)
                    return scp

                def tail(sk, scp):
                    j = sk - NJ * qb
                    c0 = max(0, j) * P
                    ex = e_pool.tile([P, 1024], F32R, tag="e1024", name="ex")
                    for hi in range(2):
                        nc.scalar.activation(
                            ex[:, hi * 512 + c0 : hi * 512 + 512],
                            scp[hi][:, c0:512],
                            mybir.ActivationFunctionType.Exp,
                            scale=0.125,
                        )
                    if j >= 0:  # diagonal strip [c0, c0+128): triangular mask
                        m128 = mask_t[:, 0:P]
                        for hi in range(2):
                            nc.vector.tensor_mul(
                                ex[:, hi * 512 + c0 : hi * 512 + c0 + P],
                                ex[:, hi * 512 + c0 : hi * 512 + c0 + P],
                                m128,
                            )
                    for hi, h in enumerate((hA, hB)):
                        nc.tensor.matmul(
                            av_ps[hi][:, c0:512],
                            r(v_t[sk][:, h * 65 : h * 65 + 65]),
                            r(ex[:, hi * 512 + c0 : hi * 512 + 512]),
                            start=(sk == 0),
                            stop=(sk == nsk - 1),
                        )

                pend = None
                for sk in range(nsk):
                    scp = scores(sk)
                    if pend is not None:
                        tail(*pend)
                    pend = (sk, scp)
                tail(*pend)

                # softmax normalization. First a single DVE copy drains av_ps
                # to SBUF so the PSUM accumulator bank frees immediately (the
                # next q-block's av matmuls reuse it); the rest of the chain
                # (GpSimd partition broadcast of the denominator row, approx
                # reciprocal, multiply) trails off the critical path.
                for hi in range(2):
                    avu = o_pool.tile([P, 512], F32, tag="outp", name="avu")
                    nc.vector.tensor_copy(avu[0:65, :], av_ps[hi][:, :])
                    bcd = o_pool.tile([P, 512], F32, tag="outp", name="bcd")
                    nc.gpsimd.partition_broadcast(bcd[0:64, :], avu[64:65, :])
                    rcb = o_pool.tile([P, 512], F32, tag="outp", name="rcb")
                    nc.vector.reciprocal_approx_fast(out=rcb[0:64, :], in_=bcd[0:64, :])
                    if hi == 0:
                        nc.vector.tensor_mul(
                            avT[p][0:64, Q0 : Q0 + QB], avu[0:64, :], rcb[0:64, :]
                        )
                    else:
                        tmp = o_pool.tile([P, 512], F32R, tag="outp")
                        nc.vector.tensor_mul(tmp[0:64, :], avu[0:64, :], rcb[0:64, :])
                        # partition shift 0:64 -> 64:128 (engines can't cross lanes)
                        nc.sync.dma_start(avT[p][64:128, Q0 : Q0 + QB], tmp[0:64, :])

        if debug:
            nc.sync.dma_start(dbg_qT[:, :], qT[0][:])
            nc.sync.dma_start(dbg_kT[:, :], kT[0][:])
            nc.sync.dma_start(dbg_v[:, :], v_t[0][:])
            nc.sync.dma_start(dbg_avT[:, :], avT[0][:])

        # ---- output projection: out[s, n] = sum_p avT[p].T @ woT[p] ----
        NW = min(512, DOUT)
        for st in range(NS):
            for nb in range(DOUT // NW):
                ps = psum_pool.tile([P, 512], F32, tag="ps", name="ps_o", bufs=2)
                for p in range(HP):
                    nc.tensor.matmul(
                        ps[:, :NW],
                        r(avT[p][:, st * P : (st + 1) * P]),
                        r(woT_t[p][:, nb * NW : (nb + 1) * NW]),
                        start=(p == 0),
                        stop=(p == HP - 1),
                    )
                ot = o_pool.tile([P, 512], F32, tag="outp", name="ot")
                nc.vector.tensor_copy(ot[:, :NW], ps[:, :NW])
                nc.sync.dma_start(
                    out[st * P : (st + 1) * P, nb * NW : (nb + 1) * NW], ot[:, :NW]
                )

    nc.compile()
    return nc


def shard_inputs(x, wq, wk, wv, wo, n_cores=8):
    """Full inputs -> per-core in_maps. Core c: batch c//2, head-group c%2."""
    B = x.shape[0]
    D = wq.shape[1]
    hg_w = wq.shape[0] // (n_cores // B)
    masks = make_masks()
    in_maps = []
    for c in range(n_cores):
        b, hg = c // (n_cores // B), c % (n_cores // B)
        sl = slice(hg * hg_w, (hg + 1) * hg_w)
        in_maps.append(
            {
                "xT": np.ascontiguousarray(x[b].T),
                "wqT": np.ascontiguousarray(wq[sl, :].T),
                "wkT": np.ascontiguousarray(wk[sl, :].T),
                "wvT": np.ascontiguousarray(wv[sl, :].T),
                "woT": np.ascontiguousarray(wo[:, sl].T),
                "masks": masks,
            }
        )
    return in_maps


def unshard_outputs(results, B=4):
    """Per-core 'out' partials -> full [B, S, D] output (sum head-group pairs)."""
    per_b = len(results) // B
    outs = []
    for b in range(B):
        acc = results[b * per_b]["out"].astype(np.float32)
        for i in range(1, per_b):
            acc = acc + results[b * per_b + i]["out"]
        outs.append(acc)
    return np.stack(outs, axis=0)


# ---------------------------------------------------------------------------
# Full-kernel entry point: FULL inputs -> FULL output, 8 NeuronCores.
# Sharding: core c -> (batch c//2, head-group c%2). Each core computes its
# batch's attention for 8 of the 16 heads plus that head-group's slice of the
# output projection; the two partial outputs per batch are summed on the host
# (the unshard step of the tensor-parallel split of wo).
# ---------------------------------------------------------------------------

_NC_CACHE = {}


def _get_program():
    if "nc" not in _NC_CACHE:
        _NC_CACHE["nc"] = build_core_program(S=2048, D=1024, DH=512, DOUT=1024)
    return _NC_CACHE["nc"]


def kernel(x, wq, wk, wv, wo):
    from concourse import bass_utils

    x = np.asarray(x, dtype=np.float32)
    wq = np.asarray(wq, dtype=np.float32)
    wk = np.asarray(wk, dtype=np.float32)
    wv = np.asarray(wv, dtype=np.float32)
    wo = np.asarray(wo, dtype=np.float32)

    nc = _get_program()
    in_maps = shard_inputs(x, wq, wk, wv, wo, n_cores=8)
    res = bass_utils.run_bass_kernel_spmd(nc, in_maps, core_ids=list(range(8)))
    return unshard_outputs(res.results, B=x.shape[0])


# revision 13
# speedup vs baseline: 1.2409x; 1.1837x over previous
"""Per-core causal multi-head attention Bass/Tile program builder.

One core handles: batch b, one head-group (DH of the model's head dims).
Computation (all on-chip after the initial loads, fp32r matmuls):
  qT = wqT.T @ xT          [DH, S]   (head dims on partitions)
  kT = wkT.T @ xT          [DH, S]
  v  = xT.T @ wvT          [S, DH]   (+ a ones column per head for softmax denom)
  per head pair p (2 heads stacked on 128 partitions):
    scoresT[sk, q] = kT.T @ qT    (K=64 contraction per head, heads packed in
                                   row strips 0-63 / 64-127 of the PE array)
    expT = exp(0.125 * scoresT)   (ACT, one [128,1024] op for the pair)
    expT *= causal mask           (diagonal tiles only)
    avT[65, q] += [v|1].T @ expT  (row 64 accumulates the softmax denominator)
    avT[0:64] *= 1/denom          (K=1 ones-matmul broadcasts the reciprocal row)
  out[s, :] = avT.T @ woT       (accumulated over head pairs, written to HBM)
"""

from contextlib import ExitStack

import numpy as np

import concourse.bass as bass
import concourse.bacc as bacc
import concourse.mybir as mybir
import concourse.tile as tile

F32 = mybir.dt.float32
F32R = mybir.dt.float32r
BF16 = mybir.dt.bfloat16
MMDT = BF16  # dtype of every matmul operand (fp32 accumulation in PSUM)


def r(ap):
    """Matmul-operand view (no-op now that operands are stored as MMDT)."""
    return ap


def make_masks(n_j=4, qb=512, extra_ones=64):
    """[128, n_j*qb + extra_ones] fp32: causal 0/1 masks for the n_j diagonal
    block offsets, plus a strip of ones (bcast-matmul lhsT / v ones source)."""
    p = np.arange(128)[:, None]
    f = np.arange(qb)[None, :]
    cols = [((p + 128 * j) <= f).astype(np.float32) for j in range(n_j)]
    cols.append(np.ones((128, extra_ones), np.float32))
    return np.concatenate(cols, axis=1)


def build_core_program(S=2048, D=1024, DH=512, DOUT=1024, QB=512, debug=False):
    """Build the per-core Bass program. Returns nc."""
    P = 128
    HP = DH // P            # head pairs
    H = DH // 64            # heads on this core
    ND = D // P             # d tiles
    NS = S // P             # s tiles of 128
    NQB = S // QB           # q blocks
    NSB = S // 512          # s blocks of 512 (projection free blocks)
    NJ = QB // P            # diagonal offsets per q block
    MCOLS = NJ * QB + 64    # masks input width

    nc = bacc.Bacc()

    xT = nc.dram_tensor("xT", [D, S], MMDT, kind="ExternalInput")
    wqT = nc.dram_tensor("wqT", [D, DH], MMDT, kind="ExternalInput")
    wkT = nc.dram_tensor("wkT", [D, DH], MMDT, kind="ExternalInput")
    wvT = nc.dram_tensor("wvT", [D, DH], MMDT, kind="ExternalInput")
    woT = nc.dram_tensor("woT", [DH, DOUT], MMDT, kind="ExternalInput")
    masks = nc.dram_tensor("masks", [P, MCOLS], MMDT, kind="ExternalInput")
    out = nc.dram_tensor("out", [S, DOUT], F32, kind="ExternalOutput")
    if debug:
        dbg_qT = nc.dram_tensor("dbg_qT", [P, S], F32, kind="ExternalOutput")
        dbg_kT = nc.dram_tensor("dbg_kT", [P, S], F32, kind="ExternalOutput")
        dbg_v = nc.dram_tensor("dbg_v", [P, H * 65], F32, kind="ExternalOutput")
        dbg_avT = nc.dram_tensor("dbg_avT", [P, S], F32, kind="ExternalOutput")
        dbg_ex = nc.dram_tensor("dbg_ex", [P, 1024], F32, kind="ExternalOutput")

    lp = nc.allow_low_precision(reason="float32r is bitwise float32 on every non-PE datapath")
    with lp, tile.TileContext(nc) as tc, ExitStack() as ctx:
        const_pool = ctx.enter_context(tc.tile_pool(name="const", bufs=1))
        # x half-tiles and avT share slots (x is dead before avT allocates)
        big_pool = ctx.enter_context(tc.tile_pool(name="big", bufs=ND // 2 + 1))
        qk_pool = ctx.enter_context(tc.tile_pool(name="qk", bufs=2 * HP))
        v_pool = ctx.enter_context(tc.tile_pool(name="v", bufs=NS))
        w_pool = ctx.enter_context(tc.tile_pool(name="w", bufs=3 * (ND // 2) + 2))
        e_pool = ctx.enter_context(tc.tile_pool(name="e1024", bufs=2 + HP))
        o_pool = ctx.enter_context(tc.tile_pool(name="outp", bufs=5))
        # one PSUM pool, 8 banks total: ps 2 + sc 2x2 + av 2 = 8 (bc shares "ps")
        psum_pool = ctx.enter_context(tc.tile_pool(name="psum", bufs=1, space="PSUM"))

        mask_t = const_pool.tile([P, MCOLS], MMDT, tag="masks")
        nc.sync.dma_start(mask_t[:], masks[:, :])
        ones64 = mask_t[:, NJ * QB : NJ * QB + 64]  # all-ones [128, 64]

        # persistent activations
        qT = [qk_pool.tile([P, S], MMDT, tag="qk", name="qT") for _ in range(HP)]
        kT = [qk_pool.tile([P, S], MMDT, tag="qk", name="kT") for _ in range(HP)]
        v_t = [v_pool.tile([P, H * 65], MMDT, tag="v", name="v_t") for _ in range(NS)]
        woT_t = [e_pool.tile([P, DOUT], MMDT, tag="e1024", name="woT_t") for _ in range(HP)]
        for p in range(HP):
            nc.sync.dma_start(woT_t[p][:], woT[p * P : (p + 1) * P, :])

        # ---- projections, d contracted in two halves to bound SBUF ----
        HALF = ND // 2
        for half in range(2):
            x_t = [big_pool.tile([P, S], MMDT, tag="big", name="x_t") for _ in range(HALF)]
            for i in range(HALF):
                dt = half * HALF + i
                nc.sync.dma_start(x_t[i][:], xT[dt * P : (dt + 1) * P, :])

            # wv first: v-proj runs before qk-proj, so its weights must not
            # wait on wq/wk pool slots (allocation order = emission order)
            wv_t = [w_pool.tile([P, DH], MMDT, tag="w", name="wv_t") for _ in range(HALF)]
            for i in range(HALF):
                dt = half * HALF + i
                nc.sync.dma_start(wv_t[i][:], wvT[dt * P : (dt + 1) * P, :])
            wq_t = [w_pool.tile([P, DH], MMDT, tag="w", name="wq_t") for _ in range(HALF)]
            wk_t = [w_pool.tile([P, DH], MMDT, tag="w", name="wk_t") for _ in range(HALF)]
            for i in range(HALF):
                dt = half * HALF + i
                nc.sync.dma_start(wq_t[i][:], wqT[dt * P : (dt + 1) * P, :])
                nc.sync.dma_start(wk_t[i][:], wkT[dt * P : (dt + 1) * P, :])

            # v first (attention needs all of v before any pair starts):
            # out [s-tile 128, DH] -> strided per-head (64 cols + ones col)
            for st in range(NS):
                ps = psum_pool.tile([P, 512], F32, tag="ps", name="ps_v", bufs=2)[:, :DH]
                for i in range(HALF):
                    nc.tensor.matmul(
                        ps[:],
                        r(x_t[i][:, st * P : (st + 1) * P]),
                        r(wv_t[i][:]),
                        start=(i == 0),
                        stop=(i == HALF - 1),
                    )
                dst = v_t[st][:].rearrange("p (h c) -> p h c", c=65)[:, :, 0:64]
                src = ps[:].rearrange("p (h c) -> p h c", c=64)
                if half == 0:
                    nc.vector.tensor_copy(dst, src)
                else:
                    nc.vector.tensor_add(dst, dst, src)
            if half == 1:
                for st in range(NS):
                    onescol = v_t[st][:].rearrange("p (h c) -> p h c", c=65)[:, :, 64:65]
                    nc.vector.tensor_copy(
                        onescol, ones64[:, 0:H].rearrange("p (h c) -> p h c", c=1)
                    )

            # q/k pair-major so attention on pair 0 can start early
            for p in range(HP):
                for w_t, dst in ((wq_t, qT), (wk_t, kT)):
                    for sb in range(NSB):
                        ps = psum_pool.tile([P, 512], F32, tag="ps", bufs=2)
                        for i in range(HALF):
                            nc.tensor.matmul(
                                ps[:],
                                r(w_t[i][:, p * P : (p + 1) * P]),
                                r(x_t[i][:, sb * 512 : (sb + 1) * 512]),
                                start=(i == 0),
                                stop=(i == HALF - 1),
                            )
                        sl = dst[p][:, sb * 512 : (sb + 1) * 512]
                        if half == 0:
                            nc.vector.tensor_copy(sl, ps[:])
                        else:
                            nc.vector.tensor_add(sl, sl, ps[:])

        # ---- attention ----
        # Software-pipelined: scores for step sk+1 are issued to the PE FIFO
        # before the av matmuls of step sk, so the in-order PE never sits
        # behind an av matmul that is still waiting on the ACT exp.
        avT = [big_pool.tile([P, S], MMDT, tag="big", name="avT") for _ in range(HP)]
        for p in range(HP):
            hA, hB = 2 * p, 2 * p + 1
            for qb in range(NQB):
                Q0 = qb * QB
                av_ps = [psum_pool.tile([65, 512], F32, tag="av", name="av_ps", bufs=2) for _ in range(2)]
                nsk = (Q0 + QB) // P

                def scores(sk):
                    K0 = sk * P
                    j = sk - NJ * qb
                    # causal: q columns f < 128j of this block can't attend
                    # to this k tile — skip them in scores/exp/av entirely.
                    c0 = max(0, j) * P
                    scp = []
                    for hi in range(2):
                        sc = psum_pool.tile(
                            [P, 512], F32, tag=f"sc{hi}", name=f"sc{hi}", bufs=2
                        )
                        nc.tensor.matmul(
                            sc[:, c0:512],
                            r(kT[p][hi * 64 : hi * 64 + 64, K0 : K0 + P]),
                            r(qT[p][hi * 64 : hi * 64 + 64, Q0 + c0 : Q0 + QB]),
                        )
                        scp.append(sc)
                    return scp

                def tail(sk, scp):
                    j = sk - NJ * qb
                    c0 = max(0, j) * P
                    ex = e_pool.tile([P, 1024], MMDT, tag="e1024", name="ex")
                    for hi in range(2):
                        nc.scalar.activation(
                            ex[:, hi * 512 + c0 : hi * 512 + 512],
                            scp[hi][:, c0:512],
                            mybir.ActivationFunctionType.Exp,
                            scale=0.125,
                        )
                    if j >= 0:  # diagonal strip [c0, c0+128): triangular mask
                        m128 = mask_t[:, 0:P]
                        for hi in range(2):
                            nc.vector.tensor_mul(
                                ex[:, hi * 512 + c0 : hi * 512 + c0 + P],
                                ex[:, hi * 512 + c0 : hi * 512 + c0 + P],
                                m128,
                            )
                    for hi, h in enumerate((hA, hB)):
                        nc.tensor.matmul(
                            av_ps[hi][:, c0:512],
                            r(v_t[sk][:, h * 65 : h * 65 + 65]),
                            r(ex[:, hi * 512 + c0 : hi * 512 + 512]),
                            start=(sk == 0),
                            stop=(sk == nsk - 1),
                        )

                pend = None
                for sk in range(nsk):
                    scp = scores(sk)
                    if pend is not None:
                        tail(*pend)
                    pend = (sk, scp)
                tail(*pend)

                # softmax normalization. First a single DVE copy drains av_ps
                # to SBUF so the PSUM accumulator bank frees immediately (the
                # next q-block's av matmuls reuse it); the rest of the chain
                # (GpSimd partition broadcast of the denominator row, approx
                # reciprocal, multiply) trails off the critical path.
                for hi in range(2):
                    avu = o_pool.tile([P, 512], F32, tag="outp", name="avu")
                    nc.vector.tensor_copy(avu[0:65, :], av_ps[hi][:, :])
                    bcd = o_pool.tile([P, 512], F32, tag="outp", name="bcd")
                    nc.gpsimd.partition_broadcast(bcd[0:64, :], avu[64:65, :])
                    rcb = o_pool.tile([P, 512], F32, tag="outp", name="rcb")
                    nc.vector.reciprocal_approx_fast(out=rcb[0:64, :], in_=bcd[0:64, :])
                    if hi == 0:
                        nc.vector.tensor_mul(
                            avT[p][0:64, Q0 : Q0 + QB], avu[0:64, :], rcb[0:64, :]
                        )
                    else:
                        tmp = o_pool.tile([P, 512], MMDT, tag="outp")
                        nc.vector.tensor_mul(tmp[0:64, :], avu[0:64, :], rcb[0:64, :])
                        # partition shift 0:64 -> 64:128 (engines can't cross lanes)
                        nc.sync.dma_start(avT[p][64:128, Q0 : Q0 + QB], tmp[0:64, :])

        if debug:
            nc.sync.dma_start(dbg_qT[:, :], qT[0][:])
            nc.sync.dma_start(dbg_kT[:, :], kT[0][:])
            nc.sync.dma_start(dbg_v[:, :], v_t[0][:])
            nc.sync.dma_start(dbg_avT[:, :], avT[0][:])

        # ---- output projection: out[s, n] = sum_p avT[p].T @ woT[p] ----
        NW = min(512, DOUT)
        for st in range(NS):
            for nb in range(DOUT // NW):
                ps = psum_pool.tile([P, 512], F32, tag="ps", name="ps_o", bufs=2)
                for p in range(HP):
                    nc.tensor.matmul(
                        ps[:, :NW],
                        r(avT[p][:, st * P : (st + 1) * P]),
                        r(woT_t[p][:, nb * NW : (nb + 1) * NW]),
                        start=(p == 0),
                        stop=(p == HP - 1),
                    )
                ot = o_pool.tile([P, 512], F32, tag="outp", name="ot")
                nc.vector.tensor_copy(ot[:, :NW], ps[:, :NW])
                nc.sync.dma_start(
                    out[st * P : (st + 1) * P, nb * NW : (nb + 1) * NW], ot[:, :NW]
                )

    nc.compile()
    return nc


def shard_inputs(x, wq, wk, wv, wo, n_cores=8):
    """Full inputs -> per-core in_maps. Core c: batch c//2, head-group c%2."""
    import ml_dtypes

    mmnp = ml_dtypes.bfloat16
    B = x.shape[0]
    hg_w = wq.shape[0] // (n_cores // B)
    masks = make_masks().astype(mmnp)
    in_maps = []
    for c in range(n_cores):
        b, hg = c // (n_cores // B), c % (n_cores // B)
        sl = slice(hg * hg_w, (hg + 1) * hg_w)
        in_maps.append(
            {
                "xT": np.ascontiguousarray(x[b].T).astype(mmnp),
                "wqT": np.ascontiguousarray(wq[sl, :].T).astype(mmnp),
                "wkT": np.ascontiguousarray(wk[sl, :].T).astype(mmnp),
                "wvT": np.ascontiguousarray(wv[sl, :].T).astype(mmnp),
                "woT": np.ascontiguousarray(wo[:, sl].T).astype(mmnp),
                "masks": masks,
            }
        )
    return in_maps


def unshard_outputs(results, B=4):
    """Per-core 'out' partials -> full [B, S, D] output (sum head-group pairs)."""
    per_b = len(results) // B
    outs = []
    for b in range(B):
        acc = results[b * per_b]["out"].astype(np.float32)
        for i in range(1, per_b):
            acc = acc + results[b * per_b + i]["out"]
        outs.append(acc)
    return np.stack(outs, axis=0)


# ---------------------------------------------------------------------------
# Full-kernel entry point: FULL inputs -> FULL output, 8 NeuronCores.
# Sharding: core c -> (batch c//2, head-group c%2). Each core computes its
# batch's attention for 8 of the 16 heads plus that head-group's slice of the
# output projection; the two partial outputs per batch are summed on the host
# (the unshard step of the tensor-parallel split of wo).
# ---------------------------------------------------------------------------

_NC_CACHE = {}


def _get_program():
    if "nc" not in _NC_CACHE:
        _NC_CACHE["nc"] = build_core_program(S=2048, D=1024, DH=512, DOUT=1024)
    return _NC_CACHE["nc"]


def kernel(x, wq, wk, wv, wo):
    from concourse import bass_utils

    x = np.asarray(x, dtype=np.float32)
    wq = np.asarray(wq, dtype=np.float32)
    wk = np.asarray(wk, dtype=np.float32)
    wv = np.asarray(wv, dtype=np.float32)
    wo = np.asarray(wo, dtype=np.float32)

    nc = _get_program()
    in_maps = shard_inputs(x, wq, wk, wv, wo, n_cores=8)
    res = bass_utils.run_bass_kernel_spmd(nc, in_maps, core_ids=list(range(8)))
    return unshard_outputs(res.results, B=x.shape[0])


# revision 14
# speedup vs baseline: 1.7087x; 1.3770x over previous
"""Per-core causal multi-head attention Bass/Tile program for Trainium2.

One core handles: batch b, one head-group (DH of the model's head dims).
All matmul operands are bf16 (PSUM accumulation is fp32); softmax runs
without max-subtraction (scores are ~N(0,1), exp is safe in fp32).

  qT = wqT.T @ xT          [DH, S]   (head dims on partitions)
  kT = wkT.T @ xT          [DH, S]
  v  = xT.T @ wvT          [S, DH]   (+ a ones column per head for softmax denom)
  per head pair p (2 heads stacked on 128 partitions):
    scoresT[sk, q] = kT.T @ qT    (K=64 contraction per head, heads packed in
                                   row strips 0-63 / 64-127 of the PE array)
    expT = exp(0.125 * scoresT)   (ACT, causally narrowed, bf16 out)
    expT *= causal mask           (diagonal 128-strip only)
    avT[65, q] += [v|1].T @ expT  (row 64 accumulates the softmax denominator)
    avT[0:64] *= 1/denom          (GpSimd lane-broadcast + approx reciprocal)
  out[s, :] = avT.T @ woT       (accumulated over head pairs, fp32 to HBM)

The attention inner loop is ACT(exp)-paced, so the q/k projection matmuls of
the NEXT head pair are interleaved into the in-order PE queue between
attention steps — the PE does projection work while waiting for exp results.
"""

from contextlib import ExitStack

import numpy as np

import concourse.bacc as bacc
import concourse.mybir as mybir
import concourse.tile as tile

F32 = mybir.dt.float32
BF16 = mybir.dt.bfloat16
MMDT = BF16  # dtype of every matmul operand (fp32 accumulation in PSUM)


def make_masks(n_j=4, qb=512, extra_ones=64):
    """[128, n_j*qb + extra_ones] causal 0/1 masks (only the first 128 cols
    are used by the kernel) plus a strip of ones (v ones-column source)."""
    p = np.arange(128)[:, None]
    f = np.arange(qb)[None, :]
    cols = [((p + 128 * j) <= f).astype(np.float32) for j in range(n_j)]
    cols.append(np.ones((128, extra_ones), np.float32))
    return np.concatenate(cols, axis=1)


def build_core_program(S=2048, D=1024, DH=512, DOUT=1024, QB=512, debug=False):
    """Build the per-core Bass program. Returns the compiled Bacc."""
    P = 128
    HP = DH // P            # head pairs
    H = DH // 64            # heads on this core
    ND = D // P             # d tiles
    NS = S // P             # s tiles of 128
    NQB = S // QB           # q blocks
    NSB = S // 512          # s blocks of 512 (projection free blocks)
    NJ = QB // P            # diagonal offsets per q block
    MCOLS = NJ * QB + 64    # masks input width

    nc = bacc.Bacc()

    xT = nc.dram_tensor("xT", [D, S], MMDT, kind="ExternalInput")
    wqT = nc.dram_tensor("wqT", [D, DH], MMDT, kind="ExternalInput")
    wkT = nc.dram_tensor("wkT", [D, DH], MMDT, kind="ExternalInput")
    wvT = nc.dram_tensor("wvT", [D, DH], MMDT, kind="ExternalInput")
    woT = nc.dram_tensor("woT", [DH, DOUT], MMDT, kind="ExternalInput")
    masks = nc.dram_tensor("masks", [P, MCOLS], MMDT, kind="ExternalInput")
    out = nc.dram_tensor("out", [S, DOUT], F32, kind="ExternalOutput")

    lp = nc.allow_low_precision(reason="bf16 matmul operands, fp32 accumulation")
    with lp, tile.TileContext(nc) as tc, ExitStack() as ctx:
        const_pool = ctx.enter_context(tc.tile_pool(name="const", bufs=1))
        x_pool = ctx.enter_context(tc.tile_pool(name="x", bufs=ND))
        qk_pool = ctx.enter_context(tc.tile_pool(name="qk", bufs=2 * HP))
        av_pool = ctx.enter_context(tc.tile_pool(name="avt", bufs=HP))
        v_pool = ctx.enter_context(tc.tile_pool(name="v", bufs=NS))
        w_pool = ctx.enter_context(tc.tile_pool(name="w", bufs=3 * ND + 2))
        wo_pool = ctx.enter_context(tc.tile_pool(name="wo", bufs=HP))
        e_pool = ctx.enter_context(tc.tile_pool(name="ex", bufs=4))
        o_pool = ctx.enter_context(tc.tile_pool(name="outp", bufs=8))
        # PSUM, 8 banks: ps 2 + sc0 2 + sc1 2 + av 2
        psum_pool = ctx.enter_context(tc.tile_pool(name="psum", bufs=1, space="PSUM"))

        mask_t = const_pool.tile([P, MCOLS], MMDT, tag="masks")
        nc.sync.dma_start(mask_t[:], masks[:, :])
        ones64 = mask_t[:, NJ * QB : NJ * QB + 64]  # all-ones [128, 64]

        # ---- persistent tiles + loads ----
        qT = [qk_pool.tile([P, S], MMDT, tag="qk", name="qT") for _ in range(HP)]
        kT = [qk_pool.tile([P, S], MMDT, tag="qk", name="kT") for _ in range(HP)]
        avT = [av_pool.tile([P, S], MMDT, tag="avt", name="avT") for _ in range(HP)]
        v_t = [v_pool.tile([P, H * 65], MMDT, tag="v", name="v_t") for _ in range(NS)]
        x_t = [x_pool.tile([P, S], MMDT, tag="x", name="x_t") for _ in range(ND)]
        for dt in range(ND):
            nc.sync.dma_start(x_t[dt][:], xT[dt * P : (dt + 1) * P, :])
        wv_t = [w_pool.tile([P, DH], MMDT, tag="w", name="wv_t") for _ in range(ND)]
        for dt in range(ND):
            nc.sync.dma_start(wv_t[dt][:], wvT[dt * P : (dt + 1) * P, :])
        wq_t = [w_pool.tile([P, DH], MMDT, tag="w", name="wq_t") for _ in range(ND)]
        wk_t = [w_pool.tile([P, DH], MMDT, tag="w", name="wk_t") for _ in range(ND)]
        for dt in range(ND):
            nc.sync.dma_start(wq_t[dt][:], wqT[dt * P : (dt + 1) * P, :])
            nc.sync.dma_start(wk_t[dt][:], wkT[dt * P : (dt + 1) * P, :])
        woT_t = [wo_pool.tile([P, DOUT], MMDT, tag="wo", name="woT_t") for _ in range(HP)]
        for p in range(HP):
            nc.sync.dma_start(woT_t[p][:], woT[p * P : (p + 1) * P, :])

        # ---- v projection (whole S; attention needs all of v) ----
        for st in range(NS):
            ps = psum_pool.tile([P, 512], F32, tag="ps", name="ps_v", bufs=2)[:, :DH]
            for dt in range(ND):
                nc.tensor.matmul(
                    ps[:],
                    x_t[dt][:, st * P : (st + 1) * P],
                    wv_t[dt][:],
                    start=(dt == 0),
                    stop=(dt == ND - 1),
                )
            dst = v_t[st][:].rearrange("p (h c) -> p h c", c=65)[:, :, 0:64]
            nc.vector.tensor_copy(dst, ps[:].rearrange("p (h c) -> p h c", c=64))
            onescol = v_t[st][:].rearrange("p (h c) -> p h c", c=65)[:, :, 64:65]
            nc.vector.tensor_copy(
                onescol, ones64[:, 0:H].rearrange("p (h c) -> p h c", c=1)
            )

        # ---- q/k projection for one pair: a list of emit-closures so the
        # attention loop of pair p-1 can interleave them into the PE queue ----
        def qk_chunks(p):
            chunks = []
            for w_t, dst in ((wq_t, qT), (wk_t, kT)):
                for sb in range(NSB):
                    def emit(w_t=w_t, dst=dst, sb=sb):
                        ps = psum_pool.tile([P, 512], F32, tag="ps", name="ps_qk", bufs=2)
                        for dt in range(ND):
                            nc.tensor.matmul(
                                ps[:],
                                w_t[dt][:, p * P : (p + 1) * P],
                                x_t[dt][:, sb * 512 : (sb + 1) * 512],
                                start=(dt == 0),
                                stop=(dt == ND - 1),
                            )
                        nc.vector.tensor_copy(dst[p][:, sb * 512 : (sb + 1) * 512], ps[:])
                    chunks.append(emit)
            return chunks

        for c in qk_chunks(0):
            c()

        # ---- attention, pair-by-pair, with next pair's qk interleaved ----
        for p in range(HP):
            hA, hB = 2 * p, 2 * p + 1
            filler = qk_chunks(p + 1) if p + 1 < HP else []
            fill_i = 0
            steps_total = sum((qb * QB + QB) // P for qb in range(NQB))
            step_no = 0

            for qb in range(NQB):
                Q0 = qb * QB
                av_ps = [
                    psum_pool.tile([65, 512], F32, tag="av", name="av_ps", bufs=2)
                    for _ in range(2)
                ]
                nsk = (Q0 + QB) // P

                def scores(sk):
                    K0 = sk * P
                    c0 = max(0, sk - NJ * qb) * P
                    scp = []
                    for hi in range(2):
                        sc = psum_pool.tile(
                            [P, 512], F32, tag=f"sc{hi}", name=f"sc{hi}", bufs=2
                        )
                        nc.tensor.matmul(
                            sc[:, c0:512],
                            kT[p][hi * 64 : hi * 64 + 64, K0 : K0 + P],
                            qT[p][hi * 64 : hi * 64 + 64, Q0 + c0 : Q0 + QB],
                        )
                        scp.append(sc)
                    return scp

                def tail(sk, scp):
                    j = sk - NJ * qb
                    c0 = max(0, j) * P
                    ex = e_pool.tile([P, 1024], MMDT, tag="ex", name="ex")
                    for hi in range(2):
                        nc.scalar.activation(
                            ex[:, hi * 512 + c0 : hi * 512 + 512],
                            scp[hi][:, c0:512],
                            mybir.ActivationFunctionType.Exp,
                            scale=0.125,
                        )
                    if j >= 0:  # diagonal strip [c0, c0+128): triangular mask
                        for hi in range(2):
                            nc.vector.tensor_mul(
                                ex[:, hi * 512 + c0 : hi * 512 + c0 + P],
                                ex[:, hi * 512 + c0 : hi * 512 + c0 + P],
                                mask_t[:, 0:P],
                            )
                    for hi, h in enumerate((hA, hB)):
                        nc.tensor.matmul(
                            av_ps[hi][:, c0:512],
                            v_t[sk][:, h * 65 : h * 65 + 65],
                            ex[:, hi * 512 + c0 : hi * 512 + 512],
                            start=(sk == 0),
                            stop=(sk == nsk - 1),
                        )

                pend = None
                for sk in range(nsk):
                    scp = scores(sk)
                    # interleave next pair's projection work evenly
                    step_no += 1
                    want = (step_no * len(filler)) // steps_total if filler else 0
                    while fill_i < want:
                        filler[fill_i]()
                        fill_i += 1
                    if pend is not None:
                        tail(*pend)
                    pend = (sk, scp)
                tail(*pend)

                # softmax normalization: drain av_ps to SBUF at once (frees
                # the PSUM bank), then the denominator chain trails off the
                # critical path (GpSimd broadcast, approx recip, multiply).
                for hi in range(2):
                    avu = o_pool.tile([P, 512], F32, tag="outp", name="avu")
                    nc.vector.tensor_copy(avu[0:65, :], av_ps[hi][:, :])
                    bcd = o_pool.tile([P, 512], F32, tag="outp", name="bcd")
                    nc.gpsimd.partition_broadcast(bcd[0:64, :], avu[64:65, :])
                    rcb = o_pool.tile([P, 512], F32, tag="outp", name="rcb")
                    nc.vector.reciprocal_approx_fast(out=rcb[0:64, :], in_=bcd[0:64, :])
                    if hi == 0:
                        nc.vector.tensor_mul(
                            avT[p][0:64, Q0 : Q0 + QB], avu[0:64, :], rcb[0:64, :]
                        )
                    else:
                        tmp = o_pool.tile([P, 512], MMDT, tag="outp", name="tmp")
                        nc.vector.tensor_mul(tmp[0:64, :], avu[0:64, :], rcb[0:64, :])
                        # partition shift 0:64 -> 64:128 (engines can't cross lanes)
                        nc.sync.dma_start(avT[p][64:128, Q0 : Q0 + QB], tmp[0:64, :])
            while fill_i < len(filler):
                filler[fill_i]()
                fill_i += 1

        # ---- output projection: out[s, n] = sum_p avT[p].T @ woT[p] ----
        NW = min(512, DOUT)
        for st in range(NS):
            for nb in range(DOUT // NW):
                ps = psum_pool.tile([P, 512], F32, tag="ps", name="ps_o", bufs=2)
                for p in range(HP):
                    nc.tensor.matmul(
                        ps[:, :NW],
                        avT[p][:, st * P : (st + 1) * P],
                        woT_t[p][:, nb * NW : (nb + 1) * NW],
                        start=(p == 0),
                        stop=(p == HP - 1),
                    )
                ot = o_pool.tile([P, 512], F32, tag="outp", name="ot")
                nc.vector.tensor_copy(ot[:, :NW], ps[:, :NW])
                nc.sync.dma_start(
                    out[st * P : (st + 1) * P, nb * NW : (nb + 1) * NW], ot[:, :NW]
                )

    nc.compile()
    return nc


# ---------------------------------------------------------------------------
# Full-kernel entry point: FULL inputs -> FULL output, 8 NeuronCores.
# Sharding: core c -> (batch c//2, head-group c%2). Each core computes its
# batch's attention for 8 of the 16 heads plus that head-group's slice of the
# output projection; the two partial outputs per batch are summed on the host
# (the unshard step of the tensor-parallel split of wo).
# ---------------------------------------------------------------------------

_NC_CACHE = {}


def _get_program():
    if "nc" not in _NC_CACHE:
        _NC_CACHE["nc"] = build_core_program(S=2048, D=1024, DH=512, DOUT=1024)
    return _NC_CACHE["nc"]


def shard_inputs(x, wq, wk, wv, wo, n_cores=8):
    """Full inputs -> per-core in_maps. Core c: batch c//2, head-group c%2."""
    import ml_dtypes

    mmnp = ml_dtypes.bfloat16
    B = x.shape[0]
    hg_w = wq.shape[0] // (n_cores // B)
    masks = make_masks().astype(mmnp)
    in_maps = []
    for c in range(n_cores):
        b, hg = c // (n_cores // B), c % (n_cores // B)
        sl = slice(hg * hg_w, (hg + 1) * hg_w)
        in_maps.append(
            {
                "xT": np.ascontiguousarray(x[b].T).astype(mmnp),
                "wqT": np.ascontiguousarray(wq[sl, :].T).astype(mmnp),
                "wkT": np.ascontiguousarray(wk[sl, :].T).astype(mmnp),
                "wvT": np.ascontiguousarray(wv[sl, :].T).astype(mmnp),
                "woT": np.ascontiguousarray(wo[:, sl].T).astype(mmnp),
                "masks": masks,
            }
        )
    return in_maps


def unshard_outputs(results, B=4):
    """Per-core 'out' partials -> full [B, S, D] output (sum head-group pairs)."""
    per_b = len(results) // B
    outs = []
    for b in range(B):
        acc = results[b * per_b]["out"].astype(np.float32)
        for i in range(1, per_b):
            acc = acc + results[b * per_b + i]["out"]
        outs.append(acc)
    return np.stack(outs, axis=0)


def kernel(x, wq, wk, wv, wo):
    from concourse import bass_utils

    x = np.asarray(x, dtype=np.float32)
    wq = np.asarray(wq, dtype=np.float32)
    wk = np.asarray(wk, dtype=np.float32)
    wv = np.asarray(wv, dtype=np.float32)
    wo = np.asarray(wo, dtype=np.float32)

    nc = _get_program()
    in_maps = shard_inputs(x, wq, wk, wv, wo, n_cores=8)
    res = bass_utils.run_bass_kernel_spmd(nc, in_maps, core_ids=list(range(8)))
    return unshard_outputs(res.results, B=x.shape[0])


# revision 16
# speedup vs baseline: 1.7337x; 1.0146x over previous
"""Per-core causal multi-head attention Bass/Tile program for Trainium2.

One core handles: batch b, one head-group (DH of the model's head dims).
All matmul operands are bf16 (PSUM accumulation is fp32); softmax runs
without max-subtraction (scores are ~N(0,1), exp is safe in fp32).

  qT = wqT.T @ xT          [DH, S]   (head dims on partitions)
  kT = wkT.T @ xT          [DH, S]
  v  = xT.T @ wvT          [S, DH]   (+ a ones column per head for softmax denom)
  per head pair p (2 heads stacked on 128 partitions):
    scoresT[sk, q] = kT.T @ qT    (K=64 contraction per head, heads packed in
                                   row strips 0-63 / 64-127 of the PE array)
    expT = exp(0.125 * scoresT)   (ACT, causally narrowed, bf16 out)
    expT *= causal mask           (diagonal 128-strip only)
    avT[65, q] += [v|1].T @ expT  (row 64 accumulates the softmax denominator)
    avT[0:64] *= 1/denom          (GpSimd lane-broadcast + approx reciprocal)
  out[s, :] = avT.T @ woT       (accumulated over head pairs, fp32 to HBM)

The attention inner loop is ACT(exp)-paced, so the q/k projection matmuls of
the NEXT head pair are interleaved into the in-order PE queue between
attention steps — the PE does projection work while waiting for exp results.
"""

from contextlib import ExitStack

import numpy as np

import concourse.bacc as bacc
import concourse.mybir as mybir
import concourse.tile as tile

F32 = mybir.dt.float32
BF16 = mybir.dt.bfloat16
MMDT = BF16  # dtype of every matmul operand (fp32 accumulation in PSUM)


def make_masks(n_j=4, qb=512, extra_ones=64):
    """[128, n_j*qb + extra_ones] causal 0/1 masks (only the first 128 cols
    are used by the kernel) plus a strip of ones (v ones-column source)."""
    p = np.arange(128)[:, None]
    f = np.arange(qb)[None, :]
    cols = [((p + 128 * j) <= f).astype(np.float32) for j in range(n_j)]
    cols.append(np.ones((128, extra_ones), np.float32))
    return np.concatenate(cols, axis=1)


def build_core_program(S=2048, D=1024, DH=512, DOUT=1024, QB=512, debug=False):
    """Build the per-core Bass program. Returns the compiled Bacc."""
    P = 128
    HP = DH // P            # head pairs
    H = DH // 64            # heads on this core
    ND = D // P             # d tiles
    NS = S // P             # s tiles of 128
    NQB = S // QB           # q blocks
    NSB = S // 512          # s blocks of 512 (projection free blocks)
    NJ = QB // P            # diagonal offsets per q block
    MCOLS = NJ * QB + 64    # masks input width

    nc = bacc.Bacc()

    xT = nc.dram_tensor("xT", [D, S], MMDT, kind="ExternalInput")
    wqT = nc.dram_tensor("wqT", [D, DH], MMDT, kind="ExternalInput")
    wkT = nc.dram_tensor("wkT", [D, DH], MMDT, kind="ExternalInput")
    wvT = nc.dram_tensor("wvT", [D, DH], MMDT, kind="ExternalInput")
    woT = nc.dram_tensor("woT", [DH, DOUT], MMDT, kind="ExternalInput")
    masks = nc.dram_tensor("masks", [P, MCOLS], MMDT, kind="ExternalInput")
    out = nc.dram_tensor("out", [S, DOUT], F32, kind="ExternalOutput")

    lp = nc.allow_low_precision(reason="bf16 matmul operands, fp32 accumulation")
    with lp, tile.TileContext(nc) as tc, ExitStack() as ctx:
        const_pool = ctx.enter_context(tc.tile_pool(name="const", bufs=1))
        x_pool = ctx.enter_context(tc.tile_pool(name="x", bufs=ND))
        qk_pool = ctx.enter_context(tc.tile_pool(name="qk", bufs=2 * HP))
        av_pool = ctx.enter_context(tc.tile_pool(name="avt", bufs=HP))
        v_pool = ctx.enter_context(tc.tile_pool(name="v", bufs=NS))
        w_pool = ctx.enter_context(tc.tile_pool(name="w", bufs=3 * ND + 2))
        wo_pool = ctx.enter_context(tc.tile_pool(name="wo", bufs=HP))
        e_pool = ctx.enter_context(tc.tile_pool(name="ex", bufs=4))
        o_pool = ctx.enter_context(tc.tile_pool(name="outp", bufs=8))
        # PSUM, 8 banks: ps 2 + sc0 2 + sc1 2 + av 2
        psum_pool = ctx.enter_context(tc.tile_pool(name="psum", bufs=1, space="PSUM"))

        mask_t = const_pool.tile([P, MCOLS], MMDT, tag="masks")
        nc.sync.dma_start(mask_t[:], masks[:, :])
        ones64 = mask_t[:, NJ * QB : NJ * QB + 64]  # all-ones [128, 64]

        # ---- persistent tiles + loads ----
        qT = [qk_pool.tile([P, S], MMDT, tag="qk", name="qT") for _ in range(HP)]
        kT = [qk_pool.tile([P, S], MMDT, tag="qk", name="kT") for _ in range(HP)]
        avT = [av_pool.tile([P, S], MMDT, tag="avt", name="avT") for _ in range(HP)]
        v_t = [v_pool.tile([P, H * 65], MMDT, tag="v", name="v_t") for _ in range(NS)]
        x_t = [x_pool.tile([P, S], MMDT, tag="x", name="x_t") for _ in range(ND)]
        for dt in range(ND):
            nc.sync.dma_start(x_t[dt][:], xT[dt * P : (dt + 1) * P, :])
        wv_t = [w_pool.tile([P, DH], MMDT, tag="w", name="wv_t") for _ in range(ND)]
        for dt in range(ND):
            nc.sync.dma_start(wv_t[dt][:], wvT[dt * P : (dt + 1) * P, :])
        wq_t = [w_pool.tile([P, DH], MMDT, tag="w", name="wq_t") for _ in range(ND)]
        wk_t = [w_pool.tile([P, DH], MMDT, tag="w", name="wk_t") for _ in range(ND)]
        for dt in range(ND):
            nc.sync.dma_start(wq_t[dt][:], wqT[dt * P : (dt + 1) * P, :])
            nc.sync.dma_start(wk_t[dt][:], wkT[dt * P : (dt + 1) * P, :])
        woT_t = [wo_pool.tile([P, DOUT], MMDT, tag="wo", name="woT_t") for _ in range(HP)]
        for p in range(HP):
            nc.sync.dma_start(woT_t[p][:], woT[p * P : (p + 1) * P, :])

        # ---- projection emit-closures, interleaved into the attention loop
        # so the in-order PE queue has projection work to chew on while
        # waiting for ACT exp results ----
        def v_chunk(st):
            def emit():
                ps = psum_pool.tile([P, 512], F32, tag="ps", name="ps_v", bufs=2)[:, :DH]
                for dt in range(ND):
                    nc.tensor.matmul(
                        ps[:],
                        x_t[dt][:, st * P : (st + 1) * P],
                        wv_t[dt][:],
                        start=(dt == 0),
                        stop=(dt == ND - 1),
                    )
                dst = v_t[st][:].rearrange("p (h c) -> p h c", c=65)[:, :, 0:64]
                nc.vector.tensor_copy(dst, ps[:].rearrange("p (h c) -> p h c", c=64))
                onescol = v_t[st][:].rearrange("p (h c) -> p h c", c=65)[:, :, 64:65]
                nc.vector.tensor_copy(
                    onescol, ones64[:, 0:H].rearrange("p (h c) -> p h c", c=1)
                )
            return emit

        def qk_chunks(p):
            chunks = []
            for w_t, dst in ((wq_t, qT), (wk_t, kT)):
                for sb in range(NSB):
                    def emit(w_t=w_t, dst=dst, sb=sb):
                        ps = psum_pool.tile([P, 512], F32, tag="ps", name="ps_qk", bufs=2)
                        for dt in range(ND):
                            nc.tensor.matmul(
                                ps[:],
                                w_t[dt][:, p * P : (p + 1) * P],
                                x_t[dt][:, sb * 512 : (sb + 1) * 512],
                                start=(dt == 0),
                                stop=(dt == ND - 1),
                            )
                        nc.vector.tensor_copy(dst[p][:, sb * 512 : (sb + 1) * 512], ps[:])
                    chunks.append(emit)
            return chunks

        def out_chunk(st, nb):
            def emit():
                ps = psum_pool.tile([P, 512], F32, tag="ps", name="ps_o", bufs=2)
                for p_ in range(HP):
                    nc.tensor.matmul(
                        ps[:, :NW],
                        avT[p_][:, st * P : (st + 1) * P],
                        woT_t[p_][:, nb * NW : (nb + 1) * NW],
                        start=(p_ == 0),
                        stop=(p_ == HP - 1),
                    )
                ot = o_pool.tile([P, 512], F32, tag="outp", name="ot")
                nc.vector.tensor_copy(ot[:, :NW], ps[:, :NW])
                nc.sync.dma_start(
                    out[st * P : (st + 1) * P, nb * NW : (nb + 1) * NW], ot[:, :NW]
                )
            return emit

        NW = min(512, DOUT)

        # startup: the first q-block of pair 0 only needs v tiles 0..NJ-1
        for st in range(NJ):
            v_chunk(st)()
        for c in qk_chunks(0):
            c()

        # ---- attention, pair-by-pair, with deadline-scheduled fillers:
        # pair 0 streams the rest of the v projection (v tiles for q-block
        # qb+1 are due before qb+1 starts), every pair prefetches the next
        # pair's q/k projection, and pair HP-1 drains the output projection
        # for q-blocks whose avT rows are complete. ----
        for p in range(HP):
            hA, hB = 2 * p, 2 * p + 1
            filler = qk_chunks(p + 1) if p + 1 < HP else []
            fill_i = 0
            steps_total = sum((qb * QB + QB) // P for qb in range(NQB))
            step_no = 0

            for qb in range(NQB):
                if p == 0 and qb + 1 < NQB:
                    # v tiles needed by the next q-block, due before it starts
                    for st in range((qb + 1) * NJ, (qb + 2) * NJ):
                        v_chunk(st)()
                Q0 = qb * QB
                av_ps = [
                    psum_pool.tile([65, 512], F32, tag="av", name="av_ps", bufs=2)
                    for _ in range(2)
                ]
                nsk = (Q0 + QB) // P

                def scores(sk):
                    K0 = sk * P
                    c0 = max(0, sk - NJ * qb) * P
                    scp = []
                    for hi in range(2):
                        sc = psum_pool.tile(
                            [P, 512], F32, tag=f"sc{hi}", name=f"sc{hi}", bufs=2
                        )
                        nc.tensor.matmul(
                            sc[:, c0:512],
                            kT[p][hi * 64 : hi * 64 + 64, K0 : K0 + P],
                            qT[p][hi * 64 : hi * 64 + 64, Q0 + c0 : Q0 + QB],
                        )
                        scp.append(sc)
                    return scp

                def tail(sk, scp):
                    j = sk - NJ * qb
                    c0 = max(0, j) * P
                    ex = e_pool.tile([P, 1024], MMDT, tag="ex", name="ex")
                    for hi in range(2):
                        nc.scalar.activation(
                            ex[:, hi * 512 + c0 : hi * 512 + 512],
                            scp[hi][:, c0:512],
                            mybir.ActivationFunctionType.Exp,
                            scale=0.125,
                        )
                    if j >= 0:  # diagonal strip [c0, c0+128): triangular mask
                        for hi in range(2):
                            nc.vector.tensor_mul(
                                ex[:, hi * 512 + c0 : hi * 512 + c0 + P],
                                ex[:, hi * 512 + c0 : hi * 512 + c0 + P],
                                mask_t[:, 0:P],
                            )
                    for hi, h in enumerate((hA, hB)):
                        nc.tensor.matmul(
                            av_ps[hi][:, c0:512],
                            v_t[sk][:, h * 65 : h * 65 + 65],
                            ex[:, hi * 512 + c0 : hi * 512 + 512],
                            start=(sk == 0),
                            stop=(sk == nsk - 1),
                        )

                pend = None
                for sk in range(nsk):
                    scp = scores(sk)
                    # interleave next pair's projection work evenly
                    step_no += 1
                    want = (step_no * len(filler)) // steps_total if filler else 0
                    while fill_i < want:
                        filler[fill_i]()
                        fill_i += 1
                    if pend is not None:
                        tail(*pend)
                    pend = (sk, scp)
                tail(*pend)

                # softmax normalization: drain av_ps to SBUF at once (frees
                # the PSUM bank), then the denominator chain trails off the
                # critical path (GpSimd broadcast, approx recip, multiply).
                for hi in range(2):
                    avu = o_pool.tile([P, 512], F32, tag="outp", name="avu")
                    nc.vector.tensor_copy(avu[0:65, :], av_ps[hi][:, :])
                    bcd = o_pool.tile([P, 512], F32, tag="outp", name="bcd")
                    nc.gpsimd.partition_broadcast(bcd[0:64, :], avu[64:65, :])
                    rcb = o_pool.tile([P, 512], F32, tag="outp", name="rcb")
                    nc.vector.reciprocal_approx_fast(out=rcb[0:64, :], in_=bcd[0:64, :])
                    if hi == 0:
                        nc.vector.tensor_mul(
                            avT[p][0:64, Q0 : Q0 + QB], avu[0:64, :], rcb[0:64, :]
                        )
                    else:
                        tmp = o_pool.tile([P, 512], MMDT, tag="outp", name="tmp")
                        nc.vector.tensor_mul(tmp[0:64, :], avu[0:64, :], rcb[0:64, :])
                        # partition shift 0:64 -> 64:128 (engines can't cross lanes)
                        nc.sync.dma_start(avT[p][64:128, Q0 : Q0 + QB], tmp[0:64, :])
                if p == HP - 1:
                    # this q-block's avT rows are final for every pair: drain
                    # the output projection for its s-tiles
                    for st in range(qb * NJ, (qb + 1) * NJ):
                        for nb in range(DOUT // NW):
                            out_chunk(st, nb)()
            while fill_i < len(filler):
                filler[fill_i]()
                fill_i += 1

    nc.compile()
    return nc


# ---------------------------------------------------------------------------
# Full-kernel entry point: FULL inputs -> FULL output, 8 NeuronCores.
# Sharding: core c -> (batch c//2, head-group c%2). Each core computes its
# batch's attention for 8 of the 16 heads plus that head-group's slice of the
# output projection; the two partial outputs per batch are summed on the host
# (the unshard step of the tensor-parallel split of wo).
# ---------------------------------------------------------------------------

_NC_CACHE = {}


def _get_program():
    if "nc" not in _NC_CACHE:
        _NC_CACHE["nc"] = build_core_program(S=2048, D=1024, DH=512, DOUT=1024)
    return _NC_CACHE["nc"]


def shard_inputs(x, wq, wk, wv, wo, n_cores=8):
    """Full inputs -> per-core in_maps. Core c: batch c//2, head-group c%2."""
    import ml_dtypes

    mmnp = ml_dtypes.bfloat16
    B = x.shape[0]
    hg_w = wq.shape[0] // (n_cores // B)
    masks = make_masks().astype(mmnp)
    in_maps = []
    for c in range(n_cores):
        b, hg = c // (n_cores // B), c % (n_cores // B)
        sl = slice(hg * hg_w, (hg + 1) * hg_w)
        in_maps.append(
            {
                "xT": np.ascontiguousarray(x[b].T).astype(mmnp),
                "wqT": np.ascontiguousarray(wq[sl, :].T).astype(mmnp),
                "wkT": np.ascontiguousarray(wk[sl, :].T).astype(mmnp),
                "wvT": np.ascontiguousarray(wv[sl, :].T).astype(mmnp),
                "woT": np.ascontiguousarray(wo[:, sl].T).astype(mmnp),
                "masks": masks,
            }
        )
    return in_maps


def unshard_outputs(results, B=4):
    """Per-core 'out' partials -> full [B, S, D] output (sum head-group pairs)."""
    per_b = len(results) // B
    outs = []
    for b in range(B):
        acc = results[b * per_b]["out"].astype(np.float32)
        for i in range(1, per_b):
            acc = acc + results[b * per_b + i]["out"]
        outs.append(acc)
    return np.stack(outs, axis=0)


def kernel(x, wq, wk, wv, wo):
    from concourse import bass_utils

    x = np.asarray(x, dtype=np.float32)
    wq = np.asarray(wq, dtype=np.float32)
    wk = np.asarray(wk, dtype=np.float32)
    wv = np.asarray(wv, dtype=np.float32)
    wo = np.asarray(wo, dtype=np.float32)

    nc = _get_program()
    in_maps = shard_inputs(x, wq, wk, wv, wo, n_cores=8)
    res = bass_utils.run_bass_kernel_spmd(nc, in_maps, core_ids=list(range(8)))
    return unshard_outputs(res.results, B=x.shape[0])
